# revision 52
# baseline (speedup 1.0000x reference)
"""Causal multi-head self-attention on 8 TRN2 NeuronCores.

Problem (hardcoded): x [4, 2048, 1024] f32, qkv_w [1024, 3072], proj_w
[1024, 1024], proj_b [1024], 16 heads of dim 64, causal softmax.

Sharding: core c handles batch b = c // 2 and head-half c % 2 (8 of the 16
heads). Each core computes the QKV projection for its 8 heads, causal
attention, and the partial output projection (its 512 rows of proj_w). The
host sums the two partials per batch and adds the bias.

All matmul operands are bf16 (accumulation in f32 PSUM): halves DMA bytes
and enables FWL so LDWEIGHTS hides under streaming. The host pre-permutes
each weight so it lands in SBUF with a single large contiguous DMA.

On-core dataflow (head-dim on partitions everywhere):
  qT/kT = W.T @ x.T  (bf16 matmuls, stored bf16)            [128, N] per pair
  v     = x @ Wv     (bf16, stored bf16, k-rows on parts)   [N, 512+ones]
  scoresT[k,q] per head = kT-slice.T @ qT   (row-tiled e0/e1 concurrent)
  expT  = exp(0.125 * scoresT) on ACT, tri-mask on the diagonal 128-block
  outT_unnorm[dh,q] += v-slice.T @ expT     (accumulated over k chunks)
  sums ride the v ones-column -> sel-matmul broadcast -> fast reciprocal
  outT  = po * recip fused on DVE (divide folded into the PSUM->SBUF copy)
  partial = outT.T @ proj_w (bf16)

Scheduling: the attention inner loop is paced by the ACT engine (exp).
One flat stream walks (pair, q-window) in a diagonal wavefront with scores
emitted one k-block ahead of exp and AV four behind it, so neither a DMA
wait nor a PSUM drain at a window tail starves the exp stream. The QKV
projections (q/k chunks and the v pass) and the output projection are
2-matmul filler closures drained into the PE bubbles under the exps, gated
only by true data dependencies (chunk before its window, v rows before
their AV k-block, projection after its window completes on all pairs).
"""

import numpy as np

P = 128
N = 2048
D = 1024
DH = 512          # head dims per core (8 heads x 64)
HD = 64
HP = HD + 1       # head dims + ones column (softmax denominator row)
DHP = 8 * HP      # per-row-chunk v columns incl. ones (520)
NPAIR = 4
DC = D // P       # 8 contraction chunks
NRC = N // P      # 16 row chunks
NQC = N // 512    # 4 query 512-chunks

_CACHE = {}


def _build_nc(reps=1):
    from collections import deque
    from contextlib import ExitStack

    import concourse.bacc as bacc
    import concourse.tile as tile
    from concourse import mybir

    f32 = mybir.dt.float32
    bf16 = mybir.dt.bfloat16
    AF = mybir.ActivationFunctionType

    nc = bacc.Bacc("TRN2", target_bir_lowering=False, debug=False,
                   enable_asserts=False, num_devices=8)

    # host-side layouts (see make_in_maps):
    #   xt  [128, 16 * 1024]: block (qw, dc) of 512 cols = x.T[dc-rows, qw-cols]
    #   wv/wq/wk [128, 8 * 512]: block dc = W[dc-rows, :]
    #   pw  [128, 4 * 1024]: block pp = proj_w[pp-rows, :]
    xt = nc.dram_tensor("xt", [P, 4 * 4096], bf16, kind="ExternalInput").ap()
    wq = nc.dram_tensor("wq", [P, DC * DH], bf16, kind="ExternalInput").ap()
    wk = nc.dram_tensor("wk", [P, DC * DH], bf16, kind="ExternalInput").ap()
    wv = nc.dram_tensor("wv", [P, DC * DH], bf16, kind="ExternalInput").ap()
    pw = nc.dram_tensor("pw", [P, NPAIR * D], bf16, kind="ExternalInput").ap()
    tri = nc.dram_tensor("tri", [P, P], bf16, kind="ExternalInput").ap()
    sel = nc.dram_tensor("sel", [P, P], bf16, kind="ExternalInput").ap()
    out = nc.dram_tensor("out", [N, D], f32, kind="ExternalOutput").ap()

    def emit_rep(tc, const_tiles):
        tri_b, sel_b = const_tiles
        with ExitStack() as rep:
            big_ps = rep.enter_context(
                tc.tile_pool(name="big_ps", bufs=2, space="PSUM"))
            small_ps = rep.enter_context(
                tc.tile_pool(name="small_ps", bufs=2, space="PSUM"))
            po_ps = rep.enter_context(
                tc.tile_pool(name="po_ps", bufs=1, space="PSUM"))
            outT_pool = rep.enter_context(tc.tile_pool(name="outT", bufs=4))
            outT = [outT_pool.tile([P, N], bf16, name=f"outT{p}", tag="outT")
                    for p in range(NPAIR)]
            pw_pool = rep.enter_context(tc.tile_pool(name="pw", bufs=1))
            osb = rep.enter_context(tc.tile_pool(name="osb", bufs=4))
            pw_sb = pw_pool.tile([P, NPAIR * D], bf16)

            with ExitStack() as mid:
                xt_pool = mid.enter_context(tc.tile_pool(name="xt", bufs=1))
                xt_sb = xt_pool.tile([P, 4 * 4096], bf16)

                def xts(dc, q0, w):
                    # columns q0:q0+w of x.T row-block dc; must not cross a
                    # 512-col boundary
                    qw, r = q0 // 512, q0 % 512
                    base = qw * 4096 + dc * 512 + r
                    return xt_sb[:, base:base + w]

                vt_pool = mid.enter_context(tc.tile_pool(name="vt", bufs=1))
                # per head: 64 v-columns + a ones column, so the AV matmul's
                # 65th output partition accumulates the softmax denominator
                v_sb = vt_pool.tile([P, NRC * DHP], bf16)
                w_pool = mid.enter_context(tc.tile_pool(name="wqkv", bufs=3))
                wv_sb = w_pool.tile([P, DC * DH], bf16, tag="wv")
                wq_sb = w_pool.tile([P, DC * DH], bf16, tag="wq")
                wk_sb = w_pool.tile([P, DC * DH], bf16, tag="wk")

                # weights on the sync queue, xt on the Act queue, pw/consts
                # on the gpsimd queue. The first pieces are fine-grained so
                # phase B2's first matmuls wait on ~128KB, not ~1MB.
                # inputs split across the SP and Act DMA queues (~160GB/s
                # each), ordered by when phase B2 / the qkT chunks need them
                # wq/wk are pair-major on the host, so each pair's slice is
                # one small contiguous DMA, ordered by when the pipeline
                # needs it (pair-0 qkT chunks run almost immediately)
                # The Act queue gets only the first xt pieces: the exp
                # instructions dispatch behind these triggers in queue
                # order, and triggers for too many big pieces block on DGE
                # ring space for tens of us.
                nc.sync.dma_start(wq_sb[:, 0:1024], wq[:, 0:1024])
                nc.sync.dma_start(wk_sb[:, 0:1024], wk[:, 0:1024])
                for dc in range(DC):
                    nc.scalar.dma_start(
                        xt_sb[:, dc * 512:(dc + 1) * 512],
                        xt[:, dc * 512:(dc + 1) * 512])
                nc.scalar.dma_start(xt_sb[:, 4096:8192], xt[:, 4096:8192])
                for pp in range(1, 3):
                    nc.sync.dma_start(wq_sb[:, pp * 1024:(pp + 1) * 1024],
                                      wq[:, pp * 1024:(pp + 1) * 1024])
                    nc.sync.dma_start(wk_sb[:, pp * 1024:(pp + 1) * 1024],
                                      wk[:, pp * 1024:(pp + 1) * 1024])
                nc.sync.dma_start(wv_sb[:], wv)
                nc.sync.dma_start(wq_sb[:, 3 * 1024:4 * 1024],
                                  wq[:, 3 * 1024:4 * 1024])
                nc.sync.dma_start(wk_sb[:, 3 * 1024:4 * 1024],
                                  wk[:, 3 * 1024:4 * 1024])
                nc.sync.dma_start(xt_sb[:, 8192:12288], xt[:, 8192:12288])
                nc.sync.dma_start(xt_sb[:, 12288:16384],
                                  xt[:, 12288:16384])
                nc.gpsimd.dma_start(pw_sb[:], pw)

                # ---- Phase B2 (v = x @ Wv, k-rows on partitions) is not a
                # prefix phase: it's a queue of filler closures drained
                # inside pair-0's attention, gated so v row-chunk rc=kc is
                # emitted before the AV matmuls that read it
                b2q = deque()

                def b2_closures(rc):
                    state = {}

                    def piece(d0):
                        def go():
                            if d0 == 0:
                                state["pv"] = small_ps.tile(
                                    [P, DH], f32, name="pv", tag="sp")
                            for dc in range(d0, d0 + 2):
                                nc.tensor.matmul(
                                    state["pv"][:],
                                    xts(dc, rc * P, P),
                                    wv_sb[:, dc * DH:(dc + 1) * DH],
                                    start=(dc == 0), stop=(dc == DC - 1),
                                    skip_group_check=True)
                            if d0 == DC - 2:
                                dst = v_sb[:, rc * DHP:(rc + 1) * DHP]\
                                    .rearrange("p (h c) -> p h c", h=8)
                                nc.vector.tensor_copy(
                                    dst[:, :, 0:HD],
                                    state["pv"][:].rearrange(
                                        "p (h c) -> p h c", h=8))
                                nc.vector.memset(dst[:, :, HD:HP], 1.0)
                        return go

                    return [piece(d0) for d0 in range(0, DC, 2)]

                for rc in range(NRC):
                    b2q.extend(b2_closures(rc))

                def ensure_b2(rc):
                    # emit v chunks up to row-chunk rc before AVs need them
                    while len(b2q) > 4 * (NRC - 1 - rc):
                        b2q.popleft()()

                qkT = mid.enter_context(tc.tile_pool(name="qkT", bufs=4))
                expp = mid.enter_context(tc.tile_pool(name="expp", bufs=7))
                ssbp = mid.enter_context(tc.tile_pool(name="ssb", bufs=2))
                drc_pool = mid.enter_context(tc.tile_pool(name="drc", bufs=1))
                # two persistent divisor-staging tiles; rows 0/32 are
                # rewritten with the raw softmax sums each round, other rows
                # only need to be non-NaN for the sel matmul
                ssb_t = []
                for i in range(2):
                    t2 = ssbp.tile([P, 512], bf16, name=f"ssbt{i}", tag="ssb")
                    nc.vector.memset(t2[0:HD, :], 1.0)
                    ssb_t.append(t2)

                def qkT_chunk_closures(p, qc, wt, dstT):
                    """One 512-col chunk of the q or k projection for pair p,
                    split into four 2-matmul filler closures (~430ns each,
                    matching the PE bubble under one exp; the last one also
                    casts PSUM -> SBUF)."""
                    state = {}

                    def piece(d0):
                        def go():
                            if d0 == 0:
                                state["ps"] = small_ps.tile(
                                    [P, 512], f32, name="fqk", tag="sp")
                            for dc in range(d0, d0 + 2):
                                nc.tensor.matmul(
                                    state["ps"][:],
                                    wt[:, p * 1024 + dc * P:
                                       p * 1024 + (dc + 1) * P],
                                    xts(dc, qc * 512, 512),
                                    start=(dc == 0), stop=(dc == DC - 1),
                                    skip_group_check=True)
                            if d0 == DC - 2:
                                nc.vector.tensor_copy(
                                    dstT[:, qc * 512:(qc + 1) * 512],
                                    state["ps"][:])
                        return go

                    return [piece(d0) for d0 in range(0, DC, 2)]

                def proj_closures(rc, cc):
                    """Output projection for one [128, 512] tile: 4
                    accumulation matmuls + PSUM cast + store, as two
                    2-matmul closures."""
                    state = {}

                    def piece(p0):
                        def go():
                            if p0 == 0:
                                state["pr"] = small_ps.tile(
                                    [P, 512], f32, name="pr", tag="sp")
                            for pp in range(p0, p0 + 2):
                                nc.tensor.matmul(
                                    state["pr"][:],
                                    outT[pp][:, rc * P:(rc + 1) * P],
                                    pw_sb[:, pp * D + cc * 512:
                                          pp * D + (cc + 1) * 512],
                                    start=(pp == 0), stop=(pp == NPAIR - 1),
                                    skip_group_check=True)
                            if p0 == 2:
                                ot = osb.tile([P, 512], f32, name="ot",
                                              tag="osb")
                                nc.scalar.copy(ot[:], state["pr"][:])
                                nc.gpsimd.dma_start(
                                    out[rc * P:(rc + 1) * P,
                                        cc * 512:(cc + 1) * 512], ot[:])
                        return go

                    return [piece(0), piece(2)]

                # ---- Attention: one flat global stream in window-rotated
                # order (p0,w0),(p1,w0),...,(p3,w0),(p0,w1),...  The first
                # four windows need only the first xt piece plus the pair
                # weights, so the exp stream starts while the rest of the
                # input is still in flight and the later DMA pieces hide
                # under it. Scores run one step ahead and AV two steps
                # behind the exp stream, so a blocked AV (waiting for the
                # po bank to drain at a window tail) never starves the ACT
                # engine. qkT chunks and B2 v-chunks are gated fillers, and
                # window w's output projection follows its last pair,
                # filling later PE bubbles.
                qkt = {p: (qkT.tile([P, N], bf16, name=f"qT{p}", tag="qT"),
                           qkT.tile([P, N], bf16, name=f"kT{p}", tag="kT"))
                       for p in range(NPAIR)}
                # chunk list in the same wavefront order the windows are
                # visited: window (p, w) only adds its own chunk (p, qc=w)
                chunks = []
                ck_target = {}
                for s in range(NPAIR + NQC - 1):
                    for p in range(min(s, NPAIR - 1), -1, -1):
                        w = s - p
                        if not 0 <= w < NQC:
                            continue
                        chunks.extend(
                            qkT_chunk_closures(p, w, wq_sb, qkt[p][0]))
                        chunks.extend(
                            qkT_chunk_closures(p, w, wk_sb, qkt[p][1]))
                        ck_target[(p, w)] = len(chunks)
                ck_ptr = [0]

                def ensure_chunks(p, w):
                    while ck_ptr[0] < ck_target[(p, w)]:
                        chunks[ck_ptr[0]]()
                        ck_ptr[0] += 1

                # diagonal wavefront over (pair, window): each step opens
                # either a new pair (qkT chunk cost) or a new window (new
                # xt piece), spreading the gated filler demand evenly
                gseq = [(p, w, kc)
                        for s in range(NPAIR + NQC - 1)
                        for p in range(min(s, NPAIR - 1), -1, -1)
                        if 0 <= (w := s - p) < NQC
                        for kc in range(4 * w + 4)]
                fillq = deque()
                po = [None, None]
                tog = [0]

                def pop_fill(n=1):
                    # alternate the b2 and qkT-chunk streams, then the
                    # output projection backlog
                    for _ in range(n):
                        tog[0] ^= 1
                        if b2q and (tog[0] or ck_ptr[0] >= len(chunks)):
                            b2q.popleft()()
                        elif ck_ptr[0] < len(chunks):
                            chunks[ck_ptr[0]]()
                            ck_ptr[0] += 1
                        elif fillq:
                            fillq.popleft()()

                def emit_scores(p, qc4, kc):
                    qT, kT = qkt[p]
                    qoff = max(0, kc * P - qc4 * 512)
                    q0 = qc4 * 512 + qoff
                    q1 = (qc4 + 1) * 512
                    ps_s = big_ps.tile([P, 1024], f32, name="ps_s", tag="bp")
                    for e in range(2):
                        nc.tensor.matmul(
                            ps_s[:, e * 512 + qoff: e * 512 + 512],
                            kT[e * HD:(e + 1) * HD, kc * P:(kc + 1) * P],
                            qT[e * HD:(e + 1) * HD, q0:q1],
                            start=True, stop=True)
                    return ps_s, qoff

                def do_av(et, qoff, p, qc4, kc):
                    nkc = 4 * qc4 + 4
                    ensure_b2(kc)
                    # filler before the AV pair: a window's first AV waits
                    # for the po bank to drain, and the in-order PE queue
                    # would otherwise idle behind it
                    pop_fill(1)
                    if kc == 0:
                        po[0] = po_ps.tile([HP, 512], f32, name="po0",
                                           tag="po0")
                        po[1] = po_ps.tile([HP, 512], f32, name="po1",
                                           tag="po1")
                    for e in range(2):
                        h = 2 * p + e
                        nc.tensor.matmul(
                            po[e][0:HP, qoff:512],
                            v_sb[:, kc * DHP + h * HP:
                                 kc * DHP + (h + 1) * HP],
                            et[:, e * 512 + qoff: e * 512 + 512],
                            start=(kc == 0), stop=(kc == nkc - 1),
                            skip_group_check=True)
                    if kc == nkc - 1:
                        # q-window tail: drain po quickly (sums + raw
                        # copies), then the divisor chain and one in-place
                        # normalize of the outT slice
                        qs = slice(qc4 * 512, (qc4 + 1) * 512)
                        ssb = ssb_t[(p * NQC + qc4) % 2]
                        nc.vector.tensor_copy(ssb[0:1, :], po[0][HD:HP, :])
                        nc.vector.tensor_copy(ssb[32:33, :], po[1][HD:HP, :])
                        nc.vector.tensor_copy(outT[p][0:HD, qs],
                                              po[0][0:HD, :])
                        nc.vector.tensor_copy(outT[p][HD:P, qs],
                                              po[1][0:HD, :])
                        dps = small_ps.tile([P, 512], f32, name="dps",
                                            tag="sp")
                        nc.tensor.matmul(dps[:], sel_b[0:33, :],
                                         ssb[0:33, :], start=True, stop=True)
                        drc = drc_pool.tile([P, 512], f32, tag="drc")
                        nc.vector.reciprocal_approx_fast(drc[:], dps[:])
                        nc.vector.tensor_mul(outT[p][:, qs],
                                             outT[p][:, qs], drc[:])
                        if p == NPAIR - 1:
                            for rc in range(4 * qc4, 4 * qc4 + 4):
                                for cc in range(2):
                                    fillq.extend(proj_closures(rc, cc))
                        pop_fill(1)

                ensure_chunks(0, 0)
                sc_fifo = deque([emit_scores(*gseq[0])])
                pend = deque()
                for gi, it in enumerate(gseq):
                    p, qc4, kc = it
                    if gi + 1 < len(gseq):
                        np_, nqc4, nkc_ = gseq[gi + 1]
                        if nkc_ == 0:
                            ensure_chunks(np_, nqc4)
                        sc_fifo.append(emit_scores(*gseq[gi + 1]))
                    ps_s, qoff = sc_fifo.popleft()
                    et = expp.tile([P, 1024], bf16, name="et", tag="et")
                    ev = et[:].rearrange(
                        "p (h q) -> p h q", h=2)[:, :, qoff:512]
                    pv_ = ps_s[:].rearrange(
                        "p (h q) -> p h q", h=2)[:, :, qoff:512]
                    nc.scalar.activation(ev, pv_, AF.Exp, scale=0.125)
                    if kc >= 4 * qc4:  # diagonal block: causal mask
                        em = et[:].rearrange("p (a q) -> p a q", a=2)[
                            :, :, qoff:qoff + P]
                        trib = tri_b[:].rearrange(
                            "p (a q) -> p a q", a=1).broadcast_to([P, 2, P])
                        nc.gpsimd.tensor_mul(em, em, trib)
                    pend.append((et, qoff) + it)
                    if len(pend) > 4:
                        do_av(*pend.popleft())
                while pend:
                    do_av(*pend.popleft())
                while b2q:
                    b2q.popleft()()
                while ck_ptr[0] < len(chunks):
                    chunks[ck_ptr[0]]()
                    ck_ptr[0] += 1
                while fillq:
                    fillq.popleft()()

    with tile.TileContext(nc) as tc, ExitStack() as ctx:
        const = ctx.enter_context(tc.tile_pool(name="const", bufs=1))
        tri_b = const.tile([P, P], bf16)
        nc.gpsimd.dma_start(tri_b[:], tri)
        sel_b = const.tile([P, P], bf16)
        nc.gpsimd.dma_start(sel_b[:], sel)
        const_tiles = (tri_b, sel_b)
        for _rep in range(reps):
            emit_rep(tc, const_tiles)

    nc.compile()
    return nc


def get_nc(reps=1):
    key = f"nc{reps}"
    if key not in _CACHE:
        _CACHE[key] = _build_nc(reps=reps)
    return _CACHE[key]


def _make_runner(nc, n_cores=8):
    """Cached jit over the bass_exec primitive (mirrors
    bass2jax.run_bass_via_pjrt's multi-core path, but reusable across calls
    so jax does not re-trace per invocation)."""
    import jax
    from jax.sharding import Mesh, PartitionSpec
    from jax.experimental.shard_map import shard_map
    from concourse import bass2jax, mybir

    bass2jax.install_neuronx_cc_hook()
    part_name = nc.partition_id_tensor.name if nc.partition_id_tensor else None
    in_names, out_names, out_avals, zero_templates = [], [], [], []
    for alloc in nc.m.functions[0].allocations:
        if not isinstance(alloc, mybir.MemoryLocationSet):
            continue
        name = alloc.memorylocations[0].name
        if alloc.kind == "ExternalInput":
            if name != part_name:
                in_names.append(name)
        elif alloc.kind == "ExternalOutput":
            out_names.append(name)
            shape = tuple(alloc.tensor_shape)
            dtype = mybir.dt.np(alloc.dtype)
            out_avals.append(jax.core.ShapedArray(shape, dtype))
            zero_templates.append((shape, dtype))
    n_params = len(in_names)
    n_outs = len(out_avals)
    all_names = in_names + out_names + ([part_name] if part_name else [])

    def _body(*args):
        operands = list(args)
        if part_name:
            operands.append(bass2jax.partition_id_tensor())
        outs = bass2jax._bass_exec_p.bind(
            *operands,
            out_avals=tuple(out_avals),
            in_names=tuple(all_names),
            out_names=tuple(out_names),
            lowering_input_output_aliases=(),
            sim_require_finite=True,
            sim_require_nnan=True,
            nc=nc,
        )
        return tuple(outs)

    devices = jax.devices()[:n_cores]
    mesh = Mesh(np.asarray(devices), ("core",))
    in_specs = (PartitionSpec("core"),) * (n_params + n_outs)
    out_specs = (PartitionSpec("core"),) * n_outs
    donate = tuple(range(n_params, n_params + n_outs))
    sharded = jax.jit(
        shard_map(_body, mesh=mesh, in_specs=in_specs, out_specs=out_specs,
                  check_rep=False),
        donate_argnums=donate, keep_unused=True)

    def run(in_maps):
        concat_in = [
            np.concatenate([np.asarray(m[name]) for m in in_maps], axis=0)
            for name in in_names
        ]
        concat_zeros = [
            np.zeros((n_cores * s[0], *s[1:]), d) for s, d in zero_templates
        ]
        out_arrs = sharded(*concat_in, *concat_zeros)
        return {
            name: np.asarray(out_arrs[i]).reshape(n_cores, *zero_templates[i][0])
            for i, name in enumerate(out_names)
        }

    run.sharded = sharded
    run.mesh = mesh
    run.in_names = in_names
    run.out_names = out_names
    run.zero_templates = zero_templates
    run.n_cores = n_cores
    return run


def get_runner(reps=1):
    key = f"runner{reps}"
    if key not in _CACHE:
        _CACHE[key] = _make_runner(get_nc(reps=reps))
    return _CACHE[key]


def _fold_rows(a):
    """[8*128, C] -> [128, 8*C]: row-block dc becomes column-block dc."""
    dcn, c = a.shape[0] // P, a.shape[1]
    return np.ascontiguousarray(
        a.reshape(dcn, P, c).transpose(1, 0, 2).reshape(P, dcn * c))


def _fold_pairs(a):
    """[8*128, 4*128] -> [128, (pair, dc, 128)]: pair-major so each pair's
    projection weights are one contiguous 256KB DMA."""
    return np.ascontiguousarray(
        a.reshape(DC, P, NPAIR, P).transpose(1, 2, 0, 3).reshape(
            P, NPAIR * D))


def make_in_maps(x, qkv_w, proj_w):
    import ml_dtypes
    bf = ml_dtypes.bfloat16
    x = np.asarray(x, dtype=np.float32)
    qkv_w = np.asarray(qkv_w, dtype=bf)
    proj_w = np.asarray(proj_w, dtype=bf)
    tri = np.triu(np.ones((P, P), dtype=bf))
    sel = np.zeros((P, P), dtype=bf)
    sel[0, 0:64] = 1.0
    sel[32, 64:128] = 1.0
    in_maps = []
    for c in range(8):
        b, half = c // 2, c % 2
        hs = half * DH
        xtb = x[b].T.astype(bf)  # [1024, 2048]
        # [p, (qw dc c)]: 512-col chunk of x.T row-block dc, query window qw
        xtr = np.ascontiguousarray(
            xtb.reshape(DC, P, 4, 512).transpose(1, 2, 0, 3).reshape(
                P, 4 * 4096))
        in_maps.append({
            "xt": xtr,
            "wq": _fold_pairs(qkv_w[:, hs:hs + DH]),
            "wk": _fold_pairs(qkv_w[:, D + hs:D + hs + DH]),
            "wv": _fold_rows(qkv_w[:, 2 * D + hs:2 * D + hs + DH]),
            "pw": _fold_rows(proj_w[hs:hs + DH, :]),
            "tri": tri,
            "sel": sel,
        })
    return in_maps


def kernel(x, qkv_w, proj_w, proj_b, **_):
    proj_b = np.asarray(proj_b, dtype=np.float32)
    run = get_runner()
    in_maps = make_in_maps(x, qkv_w, proj_w)
    parts = run(in_maps)["out"]
    outp = np.empty((4, N, D), dtype=np.float32)
    for b in range(4):
        outp[b] = parts[2 * b] + parts[2 * b + 1] + proj_b[None, :]
    return outp


# revision 53
# speedup vs baseline: 1.0077x; 1.0077x over previous
"""Causal multi-head self-attention on 8 TRN2 NeuronCores.

Problem (hardcoded): x [4, 2048, 1024] f32, qkv_w [1024, 3072], proj_w
[1024, 1024], proj_b [1024], 16 heads of dim 64, causal softmax.

Sharding: core c handles batch b = c // 2 and head-half c % 2 (8 of the 16
heads). Each core computes the QKV projection for its 8 heads, causal
attention, and the partial output projection (its 512 rows of proj_w). The
host sums the two partials per batch and adds the bias.

All matmul operands are bf16 (accumulation in f32 PSUM): halves DMA bytes
and enables FWL so LDWEIGHTS hides under streaming. The host pre-permutes
each weight so it lands in SBUF with a single large contiguous DMA.

On-core dataflow (head-dim on partitions everywhere):
  qT/kT = W.T @ x.T  (bf16 matmuls, stored bf16)            [128, N] per pair
  v     = x @ Wv     (bf16, stored bf16, k-rows on parts)   [N, 512+ones]
  scoresT[k,q] per head = kT-slice.T @ qT   (row-tiled e0/e1 concurrent)
  expT  = exp(0.125 * scoresT) on ACT, tri-mask on the diagonal 128-block
  outT_unnorm[dh,q] += v-slice.T @ expT     (accumulated over k chunks)
  sums ride the v ones-column -> sel-matmul broadcast -> fast reciprocal
  outT  = po * recip fused on DVE (divide folded into the PSUM->SBUF copy)
  partial = outT.T @ proj_w (bf16)

Scheduling: the attention inner loop is paced by the ACT engine (exp).
One flat stream walks (pair, q-window) in a diagonal wavefront with scores
emitted one k-block ahead of exp and AV four behind it, so neither a DMA
wait nor a PSUM drain at a window tail starves the exp stream. The QKV
projections (q/k chunks and the v pass) and the output projection are
2-matmul filler closures drained into the PE bubbles under the exps, gated
only by true data dependencies (chunk before its window, v rows before
their AV k-block, projection after its window completes on all pairs).
"""

import numpy as np

P = 128
N = 2048
D = 1024
DH = 512          # head dims per core (8 heads x 64)
HD = 64
HP = HD + 1       # head dims + ones column (softmax denominator row)
DHP = 8 * HP      # per-row-chunk v columns incl. ones (520)
NPAIR = 4
DC = D // P       # 8 contraction chunks
NRC = N // P      # 16 row chunks
NQC = N // 512    # 4 query 512-chunks

_CACHE = {}


def _build_nc(reps=1):
    from collections import deque
    from contextlib import ExitStack

    import concourse.bacc as bacc
    import concourse.tile as tile
    from concourse import mybir

    f32 = mybir.dt.float32
    bf16 = mybir.dt.bfloat16
    AF = mybir.ActivationFunctionType

    nc = bacc.Bacc("TRN2", target_bir_lowering=False, debug=False,
                   enable_asserts=False, num_devices=8)

    # host-side layouts (see make_in_maps):
    #   xt  [128, 16 * 1024]: block (qw, dc) of 512 cols = x.T[dc-rows, qw-cols]
    #   wv/wq/wk [128, 8 * 512]: block dc = W[dc-rows, :]
    #   pw  [128, 4 * 1024]: block pp = proj_w[pp-rows, :]
    xt = nc.dram_tensor("xt", [P, 4 * 4096], bf16, kind="ExternalInput").ap()
    wq = nc.dram_tensor("wq", [P, DC * DH], bf16, kind="ExternalInput").ap()
    wk = nc.dram_tensor("wk", [P, DC * DH], bf16, kind="ExternalInput").ap()
    wv = nc.dram_tensor("wv", [P, DC * DH], bf16, kind="ExternalInput").ap()
    pw = nc.dram_tensor("pw", [P, NPAIR * D], bf16, kind="ExternalInput").ap()
    tri = nc.dram_tensor("tri", [P, P], bf16, kind="ExternalInput").ap()
    sel = nc.dram_tensor("sel", [P, P], bf16, kind="ExternalInput").ap()
    out = nc.dram_tensor("out", [N, D], f32, kind="ExternalOutput").ap()

    def emit_rep(tc, const_tiles):
        tri_b, sel_b = const_tiles
        with ExitStack() as rep:
            big_ps = rep.enter_context(
                tc.tile_pool(name="big_ps", bufs=2, space="PSUM"))
            small_ps = rep.enter_context(
                tc.tile_pool(name="small_ps", bufs=2, space="PSUM"))
            po_ps = rep.enter_context(
                tc.tile_pool(name="po_ps", bufs=1, space="PSUM"))
            outT_pool = rep.enter_context(tc.tile_pool(name="outT", bufs=4))
            outT = [outT_pool.tile([P, N], bf16, name=f"outT{p}", tag="outT")
                    for p in range(NPAIR)]
            pw_pool = rep.enter_context(tc.tile_pool(name="pw", bufs=1))
            osb = rep.enter_context(tc.tile_pool(name="osb", bufs=4))
            pw_sb = pw_pool.tile([P, NPAIR * D], bf16)

            with ExitStack() as mid:
                xt_pool = mid.enter_context(tc.tile_pool(name="xt", bufs=1))
                xt_sb = xt_pool.tile([P, 4 * 4096], bf16)

                def xts(dc, q0, w):
                    # columns q0:q0+w of x.T row-block dc; must not cross a
                    # 512-col boundary
                    qw, r = q0 // 512, q0 % 512
                    base = qw * 4096 + dc * 512 + r
                    return xt_sb[:, base:base + w]

                vt_pool = mid.enter_context(tc.tile_pool(name="vt", bufs=1))
                # per head: 64 v-columns + a ones column, so the AV matmul's
                # 65th output partition accumulates the softmax denominator
                v_sb = vt_pool.tile([P, NRC * DHP], bf16)
                w_pool = mid.enter_context(tc.tile_pool(name="wqkv", bufs=3))
                wv_sb = w_pool.tile([P, DC * DH], bf16, tag="wv")
                wq_sb = w_pool.tile([P, DC * DH], bf16, tag="wq")
                wk_sb = w_pool.tile([P, DC * DH], bf16, tag="wk")

                # weights on the sync queue, xt on the Act queue, pw/consts
                # on the gpsimd queue. The first pieces are fine-grained so
                # phase B2's first matmuls wait on ~128KB, not ~1MB.
                # inputs split across the SP and Act DMA queues (~160GB/s
                # each), ordered by when phase B2 / the qkT chunks need them
                # wq/wk are pair-major on the host, so each pair's slice is
                # one small contiguous DMA, ordered by when the pipeline
                # needs it (pair-0 qkT chunks run almost immediately)
                # The Act queue gets only the first xt pieces: the exp
                # instructions dispatch behind these triggers in queue
                # order, and triggers for too many big pieces block on DGE
                # ring space for tens of us.
                nc.sync.dma_start(wq_sb[:, 0:1024], wq[:, 0:1024])
                nc.sync.dma_start(wk_sb[:, 0:1024], wk[:, 0:1024])
                for dc in range(DC):
                    nc.scalar.dma_start(
                        xt_sb[:, dc * 512:(dc + 1) * 512],
                        xt[:, dc * 512:(dc + 1) * 512])
                nc.scalar.dma_start(xt_sb[:, 4096:8192], xt[:, 4096:8192])
                for pp in range(1, 3):
                    nc.sync.dma_start(wq_sb[:, pp * 1024:(pp + 1) * 1024],
                                      wq[:, pp * 1024:(pp + 1) * 1024])
                    nc.sync.dma_start(wk_sb[:, pp * 1024:(pp + 1) * 1024],
                                      wk[:, pp * 1024:(pp + 1) * 1024])
                nc.sync.dma_start(wv_sb[:], wv)
                nc.sync.dma_start(wq_sb[:, 3 * 1024:4 * 1024],
                                  wq[:, 3 * 1024:4 * 1024])
                nc.sync.dma_start(wk_sb[:, 3 * 1024:4 * 1024],
                                  wk[:, 3 * 1024:4 * 1024])
                nc.sync.dma_start(xt_sb[:, 8192:12288], xt[:, 8192:12288])
                nc.sync.dma_start(xt_sb[:, 12288:16384],
                                  xt[:, 12288:16384])
                nc.gpsimd.dma_start(pw_sb[:], pw)

                # ---- Phase B2 (v = x @ Wv, k-rows on partitions) is not a
                # prefix phase: it's a queue of filler closures drained
                # inside pair-0's attention, gated so v row-chunk rc=kc is
                # emitted before the AV matmuls that read it
                b2q = deque()

                def b2_closures(rc):
                    state = {}

                    def piece(d0):
                        def go():
                            if d0 == 0:
                                state["pv"] = small_ps.tile(
                                    [P, DH], f32, name="pv", tag="sp")
                            for dc in range(d0, d0 + 2):
                                nc.tensor.matmul(
                                    state["pv"][:],
                                    xts(dc, rc * P, P),
                                    wv_sb[:, dc * DH:(dc + 1) * DH],
                                    start=(dc == 0), stop=(dc == DC - 1),
                                    skip_group_check=True)
                            if d0 == DC - 2:
                                dst = v_sb[:, rc * DHP:(rc + 1) * DHP]\
                                    .rearrange("p (h c) -> p h c", h=8)
                                nc.vector.tensor_copy(
                                    dst[:, :, 0:HD],
                                    state["pv"][:].rearrange(
                                        "p (h c) -> p h c", h=8))
                                nc.vector.memset(dst[:, :, HD:HP], 1.0)
                        return go

                    return [piece(d0) for d0 in range(0, DC, 2)]

                for rc in range(NRC):
                    b2q.extend(b2_closures(rc))

                def ensure_b2(rc):
                    # emit v chunks up to row-chunk rc before AVs need them
                    while len(b2q) > 4 * (NRC - 1 - rc):
                        b2q.popleft()()

                qkT = mid.enter_context(tc.tile_pool(name="qkT", bufs=4))
                expp = mid.enter_context(tc.tile_pool(name="expp", bufs=7))
                ssbp = mid.enter_context(tc.tile_pool(name="ssb", bufs=2))
                drc_pool = mid.enter_context(tc.tile_pool(name="drc", bufs=1))
                # two persistent divisor-staging tiles; rows 0/32 are
                # rewritten with the raw softmax sums each round, other rows
                # only need to be non-NaN for the sel matmul
                ssb_t = []
                for i in range(2):
                    t2 = ssbp.tile([P, 512], bf16, name=f"ssbt{i}", tag="ssb")
                    nc.vector.memset(t2[0:HD, :], 1.0)
                    ssb_t.append(t2)

                def qkT_chunk_closures(p, qc, wt, dstT):
                    """One 512-col chunk of the q or k projection for pair p,
                    split into four 2-matmul filler closures (~430ns each,
                    matching the PE bubble under one exp; the last one also
                    casts PSUM -> SBUF)."""
                    state = {}

                    def piece(d0):
                        def go():
                            if d0 == 0:
                                state["ps"] = small_ps.tile(
                                    [P, 512], f32, name="fqk", tag="sp")
                            for dc in range(d0, d0 + 2):
                                nc.tensor.matmul(
                                    state["ps"][:],
                                    wt[:, p * 1024 + dc * P:
                                       p * 1024 + (dc + 1) * P],
                                    xts(dc, qc * 512, 512),
                                    start=(dc == 0), stop=(dc == DC - 1),
                                    skip_group_check=True)
                            if d0 == DC - 2:
                                nc.vector.tensor_copy(
                                    dstT[:, qc * 512:(qc + 1) * 512],
                                    state["ps"][:])
                        return go

                    return [piece(d0) for d0 in range(0, DC, 2)]

                def proj_closures(rc, cc):
                    """Output projection for one [128, 512] tile: 4
                    accumulation matmuls + PSUM cast + store, as two
                    2-matmul closures."""
                    state = {}

                    def piece(p0):
                        def go():
                            if p0 == 0:
                                state["pr"] = small_ps.tile(
                                    [P, 512], f32, name="pr", tag="sp")
                            for pp in range(p0, p0 + 2):
                                nc.tensor.matmul(
                                    state["pr"][:],
                                    outT[pp][:, rc * P:(rc + 1) * P],
                                    pw_sb[:, pp * D + cc * 512:
                                          pp * D + (cc + 1) * 512],
                                    start=(pp == 0), stop=(pp == NPAIR - 1),
                                    skip_group_check=True)
                            if p0 == 2:
                                ot = osb.tile([P, 512], f32, name="ot",
                                              tag="osb")
                                nc.scalar.copy(ot[:], state["pr"][:])
                                nc.gpsimd.dma_start(
                                    out[rc * P:(rc + 1) * P,
                                        cc * 512:(cc + 1) * 512], ot[:])
                        return go

                    return [piece(0), piece(2)]

                # ---- Attention: one flat global stream in window-rotated
                # order (p0,w0),(p1,w0),...,(p3,w0),(p0,w1),...  The first
                # four windows need only the first xt piece plus the pair
                # weights, so the exp stream starts while the rest of the
                # input is still in flight and the later DMA pieces hide
                # under it. Scores run one step ahead and AV two steps
                # behind the exp stream, so a blocked AV (waiting for the
                # po bank to drain at a window tail) never starves the ACT
                # engine. qkT chunks and B2 v-chunks are gated fillers, and
                # window w's output projection follows its last pair,
                # filling later PE bubbles.
                qkt = {p: (qkT.tile([P, N], bf16, name=f"qT{p}", tag="qT"),
                           qkT.tile([P, N], bf16, name=f"kT{p}", tag="kT"))
                       for p in range(NPAIR)}
                # chunk list in the same wavefront order the windows are
                # visited: window (p, w) only adds its own chunk (p, qc=w)
                chunks = []
                ck_target = {}
                for s in range(NPAIR + NQC - 1):
                    for p in range(min(s, NPAIR - 1), -1, -1):
                        w = s - p
                        if not 0 <= w < NQC:
                            continue
                        chunks.extend(
                            qkT_chunk_closures(p, w, wq_sb, qkt[p][0]))
                        chunks.extend(
                            qkT_chunk_closures(p, w, wk_sb, qkt[p][1]))
                        ck_target[(p, w)] = len(chunks)
                ck_ptr = [0]

                def ensure_chunks(p, w):
                    while ck_ptr[0] < ck_target[(p, w)]:
                        chunks[ck_ptr[0]]()
                        ck_ptr[0] += 1

                # diagonal wavefront over (pair, window): each step opens
                # either a new pair (qkT chunk cost) or a new window (new
                # xt piece), spreading the gated filler demand evenly
                gseq = [(p, w, kc)
                        for s in range(NPAIR + NQC - 1)
                        for p in range(min(s, NPAIR - 1), -1, -1)
                        if 0 <= (w := s - p) < NQC
                        for kc in range(4 * w + 4)]
                fillq = deque()
                po = [None, None]
                tog = [0]

                def pop_fill(n=1):
                    # alternate the b2 and qkT-chunk streams, then the
                    # output projection backlog
                    for _ in range(n):
                        tog[0] ^= 1
                        if b2q and (tog[0] or ck_ptr[0] >= len(chunks)):
                            b2q.popleft()()
                        elif ck_ptr[0] < len(chunks):
                            chunks[ck_ptr[0]]()
                            ck_ptr[0] += 1
                        elif fillq:
                            fillq.popleft()()

                def emit_scores(p, qc4, kc):
                    qT, kT = qkt[p]
                    qoff = max(0, kc * P - qc4 * 512)
                    q0 = qc4 * 512 + qoff
                    q1 = (qc4 + 1) * 512
                    ps_s = big_ps.tile([P, 1024], f32, name="ps_s", tag="bp")
                    for e in range(2):
                        nc.tensor.matmul(
                            ps_s[:, e * 512 + qoff: e * 512 + 512],
                            kT[e * HD:(e + 1) * HD, kc * P:(kc + 1) * P],
                            qT[e * HD:(e + 1) * HD, q0:q1],
                            start=True, stop=True)
                    return ps_s, qoff

                def do_av(et, qoff, p, qc4, kc):
                    nkc = 4 * qc4 + 4
                    ensure_b2(kc)
                    if kc == 0:
                        po[0] = po_ps.tile([HP, 512], f32, name="po0",
                                           tag="po0")
                        po[1] = po_ps.tile([HP, 512], f32, name="po1",
                                           tag="po1")
                    for e in range(2):
                        h = 2 * p + e
                        nc.tensor.matmul(
                            po[e][0:HP, qoff:512],
                            v_sb[:, kc * DHP + h * HP:
                                 kc * DHP + (h + 1) * HP],
                            et[:, e * 512 + qoff: e * 512 + 512],
                            start=(kc == 0), stop=(kc == nkc - 1),
                            skip_group_check=True)
                    # one ~430ns filler per k-block matches the PE bubble
                    # under one exp
                    pop_fill(1)
                    if kc == nkc - 1:
                        # q-window tail: drain po quickly (sums + raw
                        # copies), then the divisor chain and one in-place
                        # normalize of the outT slice
                        qs = slice(qc4 * 512, (qc4 + 1) * 512)
                        ssb = ssb_t[(p * NQC + qc4) % 2]
                        nc.vector.tensor_copy(ssb[0:1, :], po[0][HD:HP, :])
                        nc.vector.tensor_copy(ssb[32:33, :], po[1][HD:HP, :])
                        nc.vector.tensor_copy(outT[p][0:HD, qs],
                                              po[0][0:HD, :])
                        nc.vector.tensor_copy(outT[p][HD:P, qs],
                                              po[1][0:HD, :])
                        dps = small_ps.tile([P, 512], f32, name="dps",
                                            tag="sp")
                        nc.tensor.matmul(dps[:], sel_b[0:33, :],
                                         ssb[0:33, :], start=True, stop=True)
                        drc = drc_pool.tile([P, 512], f32, tag="drc")
                        nc.vector.reciprocal_approx_fast(drc[:], dps[:])
                        nc.vector.tensor_mul(outT[p][:, qs],
                                             outT[p][:, qs], drc[:])
                        if p == NPAIR - 1:
                            for rc in range(4 * qc4, 4 * qc4 + 4):
                                for cc in range(2):
                                    fillq.extend(proj_closures(rc, cc))
                        pop_fill(1)

                ensure_chunks(0, 0)
                sc_fifo = deque([emit_scores(*gseq[0])])
                pend = deque()
                for gi, it in enumerate(gseq):
                    p, qc4, kc = it
                    if gi + 1 < len(gseq):
                        np_, nqc4, nkc_ = gseq[gi + 1]
                        if nkc_ == 0:
                            ensure_chunks(np_, nqc4)
                        sc_fifo.append(emit_scores(*gseq[gi + 1]))
                    ps_s, qoff = sc_fifo.popleft()
                    et = expp.tile([P, 1024], bf16, name="et", tag="et")
                    ev = et[:].rearrange(
                        "p (h q) -> p h q", h=2)[:, :, qoff:512]
                    pv_ = ps_s[:].rearrange(
                        "p (h q) -> p h q", h=2)[:, :, qoff:512]
                    nc.scalar.activation(ev, pv_, AF.Exp, scale=0.125)
                    if kc >= 4 * qc4:  # diagonal block: causal mask
                        em = et[:].rearrange("p (a q) -> p a q", a=2)[
                            :, :, qoff:qoff + P]
                        trib = tri_b[:].rearrange(
                            "p (a q) -> p a q", a=1).broadcast_to([P, 2, P])
                        nc.gpsimd.tensor_mul(em, em, trib)
                    pend.append((et, qoff) + it)
                    if len(pend) > 4:
                        do_av(*pend.popleft())
                while pend:
                    do_av(*pend.popleft())
                while b2q:
                    b2q.popleft()()
                while ck_ptr[0] < len(chunks):
                    chunks[ck_ptr[0]]()
                    ck_ptr[0] += 1
                while fillq:
                    fillq.popleft()()

    with tile.TileContext(nc) as tc, ExitStack() as ctx:
        const = ctx.enter_context(tc.tile_pool(name="const", bufs=1))
        tri_b = const.tile([P, P], bf16)
        nc.gpsimd.dma_start(tri_b[:], tri)
        sel_b = const.tile([P, P], bf16)
        nc.gpsimd.dma_start(sel_b[:], sel)
        const_tiles = (tri_b, sel_b)
        for _rep in range(reps):
            emit_rep(tc, const_tiles)

    nc.compile()
    return nc


def get_nc(reps=1):
    key = f"nc{reps}"
    if key not in _CACHE:
        _CACHE[key] = _build_nc(reps=reps)
    return _CACHE[key]


def _make_runner(nc, n_cores=8):
    """Cached jit over the bass_exec primitive (mirrors
    bass2jax.run_bass_via_pjrt's multi-core path, but reusable across calls
    so jax does not re-trace per invocation)."""
    import jax
    from jax.sharding import Mesh, PartitionSpec
    from jax.experimental.shard_map import shard_map
    from concourse import bass2jax, mybir

    bass2jax.install_neuronx_cc_hook()
    part_name = nc.partition_id_tensor.name if nc.partition_id_tensor else None
    in_names, out_names, out_avals, zero_templates = [], [], [], []
    for alloc in nc.m.functions[0].allocations:
        if not isinstance(alloc, mybir.MemoryLocationSet):
            continue
        name = alloc.memorylocations[0].name
        if alloc.kind == "ExternalInput":
            if name != part_name:
                in_names.append(name)
        elif alloc.kind == "ExternalOutput":
            out_names.append(name)
            shape = tuple(alloc.tensor_shape)
            dtype = mybir.dt.np(alloc.dtype)
            out_avals.append(jax.core.ShapedArray(shape, dtype))
            zero_templates.append((shape, dtype))
    n_params = len(in_names)
    n_outs = len(out_avals)
    all_names = in_names + out_names + ([part_name] if part_name else [])

    def _body(*args):
        operands = list(args)
        if part_name:
            operands.append(bass2jax.partition_id_tensor())
        outs = bass2jax._bass_exec_p.bind(
            *operands,
            out_avals=tuple(out_avals),
            in_names=tuple(all_names),
            out_names=tuple(out_names),
            lowering_input_output_aliases=(),
            sim_require_finite=True,
            sim_require_nnan=True,
            nc=nc,
        )
        return tuple(outs)

    devices = jax.devices()[:n_cores]
    mesh = Mesh(np.asarray(devices), ("core",))
    in_specs = (PartitionSpec("core"),) * (n_params + n_outs)
    out_specs = (PartitionSpec("core"),) * n_outs
    donate = tuple(range(n_params, n_params + n_outs))
    sharded = jax.jit(
        shard_map(_body, mesh=mesh, in_specs=in_specs, out_specs=out_specs,
                  check_rep=False),
        donate_argnums=donate, keep_unused=True)

    def run(in_maps):
        concat_in = [
            np.concatenate([np.asarray(m[name]) for m in in_maps], axis=0)
            for name in in_names
        ]
        concat_zeros = [
            np.zeros((n_cores * s[0], *s[1:]), d) for s, d in zero_templates
        ]
        out_arrs = sharded(*concat_in, *concat_zeros)
        return {
            name: np.asarray(out_arrs[i]).reshape(n_cores, *zero_templates[i][0])
            for i, name in enumerate(out_names)
        }

    run.sharded = sharded
    run.mesh = mesh
    run.in_names = in_names
    run.out_names = out_names
    run.zero_templates = zero_templates
    run.n_cores = n_cores
    return run


def get_runner(reps=1):
    key = f"runner{reps}"
    if key not in _CACHE:
        _CACHE[key] = _make_runner(get_nc(reps=reps))
    return _CACHE[key]


def _fold_rows(a):
    """[8*128, C] -> [128, 8*C]: row-block dc becomes column-block dc."""
    dcn, c = a.shape[0] // P, a.shape[1]
    return np.ascontiguousarray(
        a.reshape(dcn, P, c).transpose(1, 0, 2).reshape(P, dcn * c))


def _fold_pairs(a):
    """[8*128, 4*128] -> [128, (pair, dc, 128)]: pair-major so each pair's
    projection weights are one contiguous 256KB DMA."""
    return np.ascontiguousarray(
        a.reshape(DC, P, NPAIR, P).transpose(1, 2, 0, 3).reshape(
            P, NPAIR * D))


def make_in_maps(x, qkv_w, proj_w):
    import ml_dtypes
    bf = ml_dtypes.bfloat16
    x = np.asarray(x, dtype=np.float32)
    qkv_w = np.asarray(qkv_w, dtype=bf)
    proj_w = np.asarray(proj_w, dtype=bf)
    tri = np.triu(np.ones((P, P), dtype=bf))
    sel = np.zeros((P, P), dtype=bf)
    sel[0, 0:64] = 1.0
    sel[32, 64:128] = 1.0
    in_maps = []
    for c in range(8):
        b, half = c // 2, c % 2
        hs = half * DH
        xtb = x[b].T.astype(bf)  # [1024, 2048]
        # [p, (qw dc c)]: 512-col chunk of x.T row-block dc, query window qw
        xtr = np.ascontiguousarray(
            xtb.reshape(DC, P, 4, 512).transpose(1, 2, 0, 3).reshape(
                P, 4 * 4096))
        in_maps.append({
            "xt": xtr,
            "wq": _fold_pairs(qkv_w[:, hs:hs + DH]),
            "wk": _fold_pairs(qkv_w[:, D + hs:D + hs + DH]),
            "wv": _fold_rows(qkv_w[:, 2 * D + hs:2 * D + hs + DH]),
            "pw": _fold_rows(proj_w[hs:hs + DH, :]),
            "tri": tri,
            "sel": sel,
        })
    return in_maps


def kernel(x, qkv_w, proj_w, proj_b, **_):
    proj_b = np.asarray(proj_b, dtype=np.float32)
    run = get_runner()
    in_maps = make_in_maps(x, qkv_w, proj_w)
    parts = run(in_maps)["out"]
    outp = np.empty((4, N, D), dtype=np.float32)
    for b in range(4):
        outp[b] = parts[2 * b] + parts[2 * b + 1] + proj_b[None, :]
    return outp


# revision 54
# speedup vs baseline: 1.0192x; 1.0114x over previous
"""Causal multi-head self-attention on 8 TRN2 NeuronCores.

Problem (hardcoded): x [4, 2048, 1024] f32, qkv_w [1024, 3072], proj_w
[1024, 1024], proj_b [1024], 16 heads of dim 64, causal softmax.

Sharding: core c handles batch b = c // 2 and head-half c % 2 (8 of the 16
heads). Each core computes the QKV projection for its 8 heads, causal
attention, and the partial output projection (its 512 rows of proj_w). The
host sums the two partials per batch and adds the bias.

All matmul operands are bf16 (accumulation in f32 PSUM): halves DMA bytes
and enables FWL so LDWEIGHTS hides under streaming. The host pre-permutes
each weight so it lands in SBUF with a single large contiguous DMA.

On-core dataflow (head-dim on partitions everywhere):
  qT/kT = W.T @ x.T  (bf16 matmuls, stored bf16)            [128, N] per pair
  v     = x @ Wv     (bf16, stored bf16, k-rows on parts)   [N, 512+ones]
  scoresT[k,q] per head = kT-slice.T @ qT   (row-tiled e0/e1 concurrent)
  expT  = exp(0.125 * scoresT) on ACT, tri-mask on the diagonal 128-block
  outT_unnorm[dh,q] += v-slice.T @ expT     (accumulated over k chunks)
  sums ride the v ones-column -> sel-matmul broadcast -> fast reciprocal
  outT  = po * recip fused on DVE (divide folded into the PSUM->SBUF copy)
  partial = outT.T @ proj_w (bf16)

Scheduling: the attention inner loop is paced by the ACT engine (exp).
One flat stream walks (pair, q-window) in a diagonal wavefront with scores
emitted one k-block ahead of exp and AV four behind it, so neither a DMA
wait nor a PSUM drain at a window tail starves the exp stream. The QKV
projections (q/k chunks and the v pass) and the output projection are
2-matmul filler closures drained into the PE bubbles under the exps, gated
only by true data dependencies (chunk before its window, v rows before
their AV k-block, projection after its window completes on all pairs).
"""

import numpy as np

P = 128
N = 2048
D = 1024
DH = 512          # head dims per core (8 heads x 64)
HD = 64
HP = HD + 1       # head dims + ones column (softmax denominator row)
DHP = 8 * HP      # per-row-chunk v columns incl. ones (520)
NPAIR = 4
DC = D // P       # 8 contraction chunks
NRC = N // P      # 16 row chunks
NQC = N // 512    # 4 query 512-chunks

_CACHE = {}


def _build_nc(reps=1):
    from collections import deque
    from contextlib import ExitStack

    import concourse.bacc as bacc
    import concourse.tile as tile
    from concourse import mybir

    f32 = mybir.dt.float32
    bf16 = mybir.dt.bfloat16
    AF = mybir.ActivationFunctionType

    nc = bacc.Bacc("TRN2", target_bir_lowering=False, debug=False,
                   enable_asserts=False, num_devices=8)

    # host-side layouts (see make_in_maps):
    #   xt  [128, 16 * 1024]: block (qw, dc) of 512 cols = x.T[dc-rows, qw-cols]
    #   wv/wq/wk [128, 8 * 512]: block dc = W[dc-rows, :]
    #   pw  [128, 4 * 1024]: block pp = proj_w[pp-rows, :]
    xt = nc.dram_tensor("xt", [P, 4 * 4096], bf16, kind="ExternalInput").ap()
    wq = nc.dram_tensor("wq", [P, DC * DH], bf16, kind="ExternalInput").ap()
    wk = nc.dram_tensor("wk", [P, DC * DH], bf16, kind="ExternalInput").ap()
    wv = nc.dram_tensor("wv", [P, DC * DH], bf16, kind="ExternalInput").ap()
    pw = nc.dram_tensor("pw", [P, NPAIR * D], bf16, kind="ExternalInput").ap()
    tri = nc.dram_tensor("tri", [P, P], bf16, kind="ExternalInput").ap()
    sel = nc.dram_tensor("sel", [P, P], bf16, kind="ExternalInput").ap()
    out = nc.dram_tensor("out", [N, D], f32, kind="ExternalOutput").ap()

    def emit_rep(tc, const_tiles):
        tri_b, sel_b = const_tiles
        with ExitStack() as rep:
            big_ps = rep.enter_context(
                tc.tile_pool(name="big_ps", bufs=2, space="PSUM"))
            small_ps = rep.enter_context(
                tc.tile_pool(name="small_ps", bufs=2, space="PSUM"))
            po_ps = rep.enter_context(
                tc.tile_pool(name="po_ps", bufs=1, space="PSUM"))
            outT_pool = rep.enter_context(tc.tile_pool(name="outT", bufs=4))
            outT = [outT_pool.tile([P, N], bf16, name=f"outT{p}", tag="outT")
                    for p in range(NPAIR)]
            pw_pool = rep.enter_context(tc.tile_pool(name="pw", bufs=1))
            osb = rep.enter_context(tc.tile_pool(name="osb", bufs=4))
            pw_sb = pw_pool.tile([P, NPAIR * D], bf16)

            with ExitStack() as mid:
                xt_pool = mid.enter_context(tc.tile_pool(name="xt", bufs=1))
                xt_sb = xt_pool.tile([P, 4 * 4096], bf16)

                def xts(dc, q0, w):
                    # columns q0:q0+w of x.T row-block dc; must not cross a
                    # 512-col boundary
                    qw, r = q0 // 512, q0 % 512
                    base = qw * 4096 + dc * 512 + r
                    return xt_sb[:, base:base + w]

                vt_pool = mid.enter_context(tc.tile_pool(name="vt", bufs=1))
                # per head: 64 v-columns + a ones column, so the AV matmul's
                # 65th output partition accumulates the softmax denominator
                v_sb = vt_pool.tile([P, NRC * DHP], bf16)
                w_pool = mid.enter_context(tc.tile_pool(name="wqkv", bufs=3))
                wv_sb = w_pool.tile([P, DC * DH], bf16, tag="wv")
                wq_sb = w_pool.tile([P, DC * DH], bf16, tag="wq")
                wk_sb = w_pool.tile([P, DC * DH], bf16, tag="wk")

                # weights on the sync queue, xt on the Act queue, pw/consts
                # on the gpsimd queue. The first pieces are fine-grained so
                # phase B2's first matmuls wait on ~128KB, not ~1MB.
                # inputs split across the SP and Act DMA queues (~160GB/s
                # each), ordered by when phase B2 / the qkT chunks need them
                # wq/wk are pair-major on the host, so each pair's slice is
                # one small contiguous DMA, ordered by when the pipeline
                # needs it (pair-0 qkT chunks run almost immediately)
                # The Act queue gets only the first xt pieces: the exp
                # instructions dispatch behind these triggers in queue
                # order, and triggers for too many big pieces block on DGE
                # ring space for tens of us.
                nc.sync.dma_start(wq_sb[:, 0:1024], wq[:, 0:1024])
                nc.sync.dma_start(wk_sb[:, 0:1024], wk[:, 0:1024])
                for dc in range(DC):
                    nc.scalar.dma_start(
                        xt_sb[:, dc * 512:(dc + 1) * 512],
                        xt[:, dc * 512:(dc + 1) * 512])
                nc.scalar.dma_start(xt_sb[:, 4096:8192], xt[:, 4096:8192])
                for pp in range(1, 3):
                    nc.sync.dma_start(wq_sb[:, pp * 1024:(pp + 1) * 1024],
                                      wq[:, pp * 1024:(pp + 1) * 1024])
                    nc.sync.dma_start(wk_sb[:, pp * 1024:(pp + 1) * 1024],
                                      wk[:, pp * 1024:(pp + 1) * 1024])
                nc.sync.dma_start(wv_sb[:], wv)
                nc.sync.dma_start(wq_sb[:, 3 * 1024:4 * 1024],
                                  wq[:, 3 * 1024:4 * 1024])
                nc.sync.dma_start(wk_sb[:, 3 * 1024:4 * 1024],
                                  wk[:, 3 * 1024:4 * 1024])
                nc.sync.dma_start(xt_sb[:, 8192:12288], xt[:, 8192:12288])
                nc.sync.dma_start(xt_sb[:, 12288:16384],
                                  xt[:, 12288:16384])
                nc.gpsimd.dma_start(pw_sb[:], pw)

                # ---- Phase B2 (v = x @ Wv, k-rows on partitions) is not a
                # prefix phase: it's a queue of filler closures drained
                # inside pair-0's attention, gated so v row-chunk rc=kc is
                # emitted before the AV matmuls that read it
                b2q = deque()

                def b2_closures(rc):
                    state = {}

                    def piece(d0):
                        def go():
                            if d0 == 0:
                                state["pv"] = small_ps.tile(
                                    [P, DH], f32, name="pv", tag="sp")
                            for dc in range(d0, d0 + 2):
                                nc.tensor.matmul(
                                    state["pv"][:],
                                    xts(dc, rc * P, P),
                                    wv_sb[:, dc * DH:(dc + 1) * DH],
                                    start=(dc == 0), stop=(dc == DC - 1),
                                    skip_group_check=True)
                            if d0 == DC - 2:
                                dst = v_sb[:, rc * DHP:(rc + 1) * DHP]\
                                    .rearrange("p (h c) -> p h c", h=8)
                                nc.vector.tensor_copy(
                                    dst[:, :, 0:HD],
                                    state["pv"][:].rearrange(
                                        "p (h c) -> p h c", h=8))
                                nc.vector.memset(dst[:, :, HD:HP], 1.0)
                        return go

                    return [piece(d0) for d0 in range(0, DC, 2)]

                for rc in range(NRC):
                    b2q.extend(b2_closures(rc))

                def ensure_b2(rc):
                    # emit v chunks up to row-chunk rc before AVs need them
                    while len(b2q) > 4 * (NRC - 1 - rc):
                        b2q.popleft()()

                qkT = mid.enter_context(tc.tile_pool(name="qkT", bufs=4))
                expp = mid.enter_context(tc.tile_pool(name="expp", bufs=8))
                ssbp = mid.enter_context(tc.tile_pool(name="ssb", bufs=2))
                drc_pool = mid.enter_context(tc.tile_pool(name="drc", bufs=1))
                # two persistent divisor-staging tiles; rows 0/32 are
                # rewritten with the raw softmax sums each round, other rows
                # only need to be non-NaN for the sel matmul
                ssb_t = []
                for i in range(2):
                    t2 = ssbp.tile([P, 512], bf16, name=f"ssbt{i}", tag="ssb")
                    nc.vector.memset(t2[0:HD, :], 1.0)
                    ssb_t.append(t2)

                def qkT_chunk_closures(p, qc, wt, dstT):
                    """One 512-col chunk of the q or k projection for pair p,
                    split into four 2-matmul filler closures (~430ns each,
                    matching the PE bubble under one exp; the last one also
                    casts PSUM -> SBUF)."""
                    state = {}

                    def piece(d0):
                        def go():
                            if d0 == 0:
                                state["ps"] = small_ps.tile(
                                    [P, 512], f32, name="fqk", tag="sp")
                            for dc in range(d0, d0 + 2):
                                nc.tensor.matmul(
                                    state["ps"][:],
                                    wt[:, p * 1024 + dc * P:
                                       p * 1024 + (dc + 1) * P],
                                    xts(dc, qc * 512, 512),
                                    start=(dc == 0), stop=(dc == DC - 1),
                                    skip_group_check=True)
                            if d0 == DC - 2:
                                nc.vector.tensor_copy(
                                    dstT[:, qc * 512:(qc + 1) * 512],
                                    state["ps"][:])
                        return go

                    return [piece(d0) for d0 in range(0, DC, 2)]

                def proj_closures(rc, cc):
                    """Output projection for one [128, 512] tile: 4
                    accumulation matmuls + PSUM cast + store, as two
                    2-matmul closures."""
                    state = {}

                    def piece(p0):
                        def go():
                            if p0 == 0:
                                state["pr"] = small_ps.tile(
                                    [P, 512], f32, name="pr", tag="sp")
                            for pp in range(p0, p0 + 2):
                                nc.tensor.matmul(
                                    state["pr"][:],
                                    outT[pp][:, rc * P:(rc + 1) * P],
                                    pw_sb[:, pp * D + cc * 512:
                                          pp * D + (cc + 1) * 512],
                                    start=(pp == 0), stop=(pp == NPAIR - 1),
                                    skip_group_check=True)
                            if p0 == 2:
                                ot = osb.tile([P, 512], f32, name="ot",
                                              tag="osb")
                                nc.scalar.copy(ot[:], state["pr"][:])
                                nc.gpsimd.dma_start(
                                    out[rc * P:(rc + 1) * P,
                                        cc * 512:(cc + 1) * 512], ot[:])
                        return go

                    return [piece(0), piece(2)]

                # ---- Attention: one flat global stream in window-rotated
                # order (p0,w0),(p1,w0),...,(p3,w0),(p0,w1),...  The first
                # four windows need only the first xt piece plus the pair
                # weights, so the exp stream starts while the rest of the
                # input is still in flight and the later DMA pieces hide
                # under it. Scores run one step ahead and AV two steps
                # behind the exp stream, so a blocked AV (waiting for the
                # po bank to drain at a window tail) never starves the ACT
                # engine. qkT chunks and B2 v-chunks are gated fillers, and
                # window w's output projection follows its last pair,
                # filling later PE bubbles.
                qkt = {p: (qkT.tile([P, N], bf16, name=f"qT{p}", tag="qT"),
                           qkT.tile([P, N], bf16, name=f"kT{p}", tag="kT"))
                       for p in range(NPAIR)}
                # chunk list in the same wavefront order the windows are
                # visited: window (p, w) only adds its own chunk (p, qc=w)
                chunks = []
                ck_target = {}
                for s in range(NPAIR + NQC - 1):
                    for p in range(min(s, NPAIR - 1), -1, -1):
                        w = s - p
                        if not 0 <= w < NQC:
                            continue
                        chunks.extend(
                            qkT_chunk_closures(p, w, wq_sb, qkt[p][0]))
                        chunks.extend(
                            qkT_chunk_closures(p, w, wk_sb, qkt[p][1]))
                        ck_target[(p, w)] = len(chunks)
                ck_ptr = [0]

                def ensure_chunks(p, w):
                    while ck_ptr[0] < ck_target[(p, w)]:
                        chunks[ck_ptr[0]]()
                        ck_ptr[0] += 1

                # diagonal wavefront over (pair, window): each step opens
                # either a new pair (qkT chunk cost) or a new window (new
                # xt piece), spreading the gated filler demand evenly
                gseq = [(p, w, kc)
                        for s in range(NPAIR + NQC - 1)
                        for p in range(min(s, NPAIR - 1), -1, -1)
                        if 0 <= (w := s - p) < NQC
                        for kc in range(4 * w + 4)]
                fillq = deque()
                po = [None, None]
                tog = [0]

                def pop_fill(n=1):
                    # alternate the b2 and qkT-chunk streams, then the
                    # output projection backlog
                    for _ in range(n):
                        tog[0] ^= 1
                        if b2q and (tog[0] or ck_ptr[0] >= len(chunks)):
                            b2q.popleft()()
                        elif ck_ptr[0] < len(chunks):
                            chunks[ck_ptr[0]]()
                            ck_ptr[0] += 1
                        elif fillq:
                            fillq.popleft()()

                def emit_scores(p, qc4, kc):
                    qT, kT = qkt[p]
                    qoff = max(0, kc * P - qc4 * 512)
                    q0 = qc4 * 512 + qoff
                    q1 = (qc4 + 1) * 512
                    ps_s = big_ps.tile([P, 1024], f32, name="ps_s", tag="bp")
                    for e in range(2):
                        nc.tensor.matmul(
                            ps_s[:, e * 512 + qoff: e * 512 + 512],
                            kT[e * HD:(e + 1) * HD, kc * P:(kc + 1) * P],
                            qT[e * HD:(e + 1) * HD, q0:q1],
                            start=True, stop=True)
                    return ps_s, qoff

                def do_av(et, qoff, p, qc4, kc):
                    nkc = 4 * qc4 + 4
                    ensure_b2(kc)
                    if kc == 0:
                        po[0] = po_ps.tile([HP, 512], f32, name="po0",
                                           tag="po0")
                        po[1] = po_ps.tile([HP, 512], f32, name="po1",
                                           tag="po1")
                    for e in range(2):
                        h = 2 * p + e
                        nc.tensor.matmul(
                            po[e][0:HP, qoff:512],
                            v_sb[:, kc * DHP + h * HP:
                                 kc * DHP + (h + 1) * HP],
                            et[:, e * 512 + qoff: e * 512 + 512],
                            start=(kc == 0), stop=(kc == nkc - 1),
                            skip_group_check=True)
                    # one ~430ns filler per k-block matches the PE bubble
                    # under one exp
                    pop_fill(1)
                    if kc == nkc - 1:
                        # q-window tail: drain po quickly (sums + raw
                        # copies), then the divisor chain and one in-place
                        # normalize of the outT slice
                        qs = slice(qc4 * 512, (qc4 + 1) * 512)
                        ssb = ssb_t[(p * NQC + qc4) % 2]
                        nc.vector.tensor_copy(ssb[0:1, :], po[0][HD:HP, :])
                        nc.vector.tensor_copy(ssb[32:33, :], po[1][HD:HP, :])
                        nc.vector.tensor_copy(outT[p][0:HD, qs],
                                              po[0][0:HD, :])
                        nc.vector.tensor_copy(outT[p][HD:P, qs],
                                              po[1][0:HD, :])
                        dps = small_ps.tile([P, 512], f32, name="dps",
                                            tag="sp")
                        nc.tensor.matmul(dps[:], sel_b[0:33, :],
                                         ssb[0:33, :], start=True, stop=True)
                        drc = drc_pool.tile([P, 512], f32, tag="drc")
                        nc.vector.reciprocal_approx_fast(drc[:], dps[:])
                        nc.vector.tensor_mul(outT[p][:, qs],
                                             outT[p][:, qs], drc[:])
                        if p == NPAIR - 1:
                            for rc in range(4 * qc4, 4 * qc4 + 4):
                                for cc in range(2):
                                    fillq.extend(proj_closures(rc, cc))
                        pop_fill(1)

                ensure_chunks(0, 0)
                sc_fifo = deque([emit_scores(*gseq[0])])
                pend = deque()
                for gi, it in enumerate(gseq):
                    p, qc4, kc = it
                    if gi + 1 < len(gseq):
                        np_, nqc4, nkc_ = gseq[gi + 1]
                        if nkc_ == 0:
                            ensure_chunks(np_, nqc4)
                        sc_fifo.append(emit_scores(*gseq[gi + 1]))
                    ps_s, qoff = sc_fifo.popleft()
                    et = expp.tile([P, 1024], bf16, name="et", tag="et")
                    ev = et[:].rearrange(
                        "p (h q) -> p h q", h=2)[:, :, qoff:512]
                    pv_ = ps_s[:].rearrange(
                        "p (h q) -> p h q", h=2)[:, :, qoff:512]
                    nc.scalar.activation(ev, pv_, AF.Exp, scale=0.125)
                    if kc >= 4 * qc4:  # diagonal block: causal mask
                        em = et[:].rearrange("p (a q) -> p a q", a=2)[
                            :, :, qoff:qoff + P]
                        trib = tri_b[:].rearrange(
                            "p (a q) -> p a q", a=1).broadcast_to([P, 2, P])
                        nc.gpsimd.tensor_mul(em, em, trib)
                    pend.append((et, qoff) + it)
                    # drain AVs in pairs: halves the number of transitions
                    # between the row-tiled score pair and the full-row AV
                    # array configuration
                    if len(pend) > 5:
                        do_av(*pend.popleft())
                        do_av(*pend.popleft())
                while pend:
                    do_av(*pend.popleft())
                while b2q:
                    b2q.popleft()()
                while ck_ptr[0] < len(chunks):
                    chunks[ck_ptr[0]]()
                    ck_ptr[0] += 1
                while fillq:
                    fillq.popleft()()

    with tile.TileContext(nc) as tc, ExitStack() as ctx:
        const = ctx.enter_context(tc.tile_pool(name="const", bufs=1))
        tri_b = const.tile([P, P], bf16)
        nc.gpsimd.dma_start(tri_b[:], tri)
        sel_b = const.tile([P, P], bf16)
        nc.gpsimd.dma_start(sel_b[:], sel)
        const_tiles = (tri_b, sel_b)
        for _rep in range(reps):
            emit_rep(tc, const_tiles)

    nc.compile()
    return nc


def get_nc(reps=1):
    key = f"nc{reps}"
    if key not in _CACHE:
        _CACHE[key] = _build_nc(reps=reps)
    return _CACHE[key]


def _make_runner(nc, n_cores=8):
    """Cached jit over the bass_exec primitive (mirrors
    bass2jax.run_bass_via_pjrt's multi-core path, but reusable across calls
    so jax does not re-trace per invocation)."""
    import jax
    from jax.sharding import Mesh, PartitionSpec
    from jax.experimental.shard_map import shard_map
    from concourse import bass2jax, mybir

    bass2jax.install_neuronx_cc_hook()
    part_name = nc.partition_id_tensor.name if nc.partition_id_tensor else None
    in_names, out_names, out_avals, zero_templates = [], [], [], []
    for alloc in nc.m.functions[0].allocations:
        if not isinstance(alloc, mybir.MemoryLocationSet):
            continue
        name = alloc.memorylocations[0].name
        if alloc.kind == "ExternalInput":
            if name != part_name:
                in_names.append(name)
        elif alloc.kind == "ExternalOutput":
            out_names.append(name)
            shape = tuple(alloc.tensor_shape)
            dtype = mybir.dt.np(alloc.dtype)
            out_avals.append(jax.core.ShapedArray(shape, dtype))
            zero_templates.append((shape, dtype))
    n_params = len(in_names)
    n_outs = len(out_avals)
    all_names = in_names + out_names + ([part_name] if part_name else [])

    def _body(*args):
        operands = list(args)
        if part_name:
            operands.append(bass2jax.partition_id_tensor())
        outs = bass2jax._bass_exec_p.bind(
            *operands,
            out_avals=tuple(out_avals),
            in_names=tuple(all_names),
            out_names=tuple(out_names),
            lowering_input_output_aliases=(),
            sim_require_finite=True,
            sim_require_nnan=True,
            nc=nc,
        )
        return tuple(outs)

    devices = jax.devices()[:n_cores]
    mesh = Mesh(np.asarray(devices), ("core",))
    in_specs = (PartitionSpec("core"),) * (n_params + n_outs)
    out_specs = (PartitionSpec("core"),) * n_outs
    donate = tuple(range(n_params, n_params + n_outs))
    sharded = jax.jit(
        shard_map(_body, mesh=mesh, in_specs=in_specs, out_specs=out_specs,
                  check_rep=False),
        donate_argnums=donate, keep_unused=True)

    def run(in_maps):
        concat_in = [
            np.concatenate([np.asarray(m[name]) for m in in_maps], axis=0)
            for name in in_names
        ]
        concat_zeros = [
            np.zeros((n_cores * s[0], *s[1:]), d) for s, d in zero_templates
        ]
        out_arrs = sharded(*concat_in, *concat_zeros)
        return {
            name: np.asarray(out_arrs[i]).reshape(n_cores, *zero_templates[i][0])
            for i, name in enumerate(out_names)
        }

    run.sharded = sharded
    run.mesh = mesh
    run.in_names = in_names
    run.out_names = out_names
    run.zero_templates = zero_templates
    run.n_cores = n_cores
    return run


def get_runner(reps=1):
    key = f"runner{reps}"
    if key not in _CACHE:
        _CACHE[key] = _make_runner(get_nc(reps=reps))
    return _CACHE[key]


def _fold_rows(a):
    """[8*128, C] -> [128, 8*C]: row-block dc becomes column-block dc."""
    dcn, c = a.shape[0] // P, a.shape[1]
    return np.ascontiguousarray(
        a.reshape(dcn, P, c).transpose(1, 0, 2).reshape(P, dcn * c))


def _fold_pairs(a):
    """[8*128, 4*128] -> [128, (pair, dc, 128)]: pair-major so each pair's
    projection weights are one contiguous 256KB DMA."""
    return np.ascontiguousarray(
        a.reshape(DC, P, NPAIR, P).transpose(1, 2, 0, 3).reshape(
            P, NPAIR * D))


def make_in_maps(x, qkv_w, proj_w):
    import ml_dtypes
    bf = ml_dtypes.bfloat16
    x = np.asarray(x, dtype=np.float32)
    qkv_w = np.asarray(qkv_w, dtype=bf)
    proj_w = np.asarray(proj_w, dtype=bf)
    tri = np.triu(np.ones((P, P), dtype=bf))
    sel = np.zeros((P, P), dtype=bf)
    sel[0, 0:64] = 1.0
    sel[32, 64:128] = 1.0
    in_maps = []
    for c in range(8):
        b, half = c // 2, c % 2
        hs = half * DH
        xtb = x[b].T.astype(bf)  # [1024, 2048]
        # [p, (qw dc c)]: 512-col chunk of x.T row-block dc, query window qw
        xtr = np.ascontiguousarray(
            xtb.reshape(DC, P, 4, 512).transpose(1, 2, 0, 3).reshape(
                P, 4 * 4096))
        in_maps.append({
            "xt": xtr,
            "wq": _fold_pairs(qkv_w[:, hs:hs + DH]),
            "wk": _fold_pairs(qkv_w[:, D + hs:D + hs + DH]),
            "wv": _fold_rows(qkv_w[:, 2 * D + hs:2 * D + hs + DH]),
            "pw": _fold_rows(proj_w[hs:hs + DH, :]),
            "tri": tri,
            "sel": sel,
        })
    return in_maps


def kernel(x, qkv_w, proj_w, proj_b, **_):
    proj_b = np.asarray(proj_b, dtype=np.float32)
    run = get_runner()
    in_maps = make_in_maps(x, qkv_w, proj_w)
    parts = run(in_maps)["out"]
    outp = np.empty((4, N, D), dtype=np.float32)
    for b in range(4):
        outp[b] = parts[2 * b] + parts[2 * b + 1] + proj_b[None, :]
    return outp
